# revision 2
# baseline (speedup 1.0000x reference)
"""ConformerEnsembleEmbeddingCombiner for Trainium2 (8 NeuronCores, SPMD).

Data-parallel over molecules: core i owns conformers [i*8192, (i+1)*8192)
(= 256 whole molecules of 32 conformers each). Weights replicated.

On-device layout is "transposed": features on the partition axis, conformers
on the free axis. The host pre-transposes the input slice (and casts to bf16)
so every MLP matmul is lhsT=W[din,dout] (stationary), rhs=x^T[din,conf]
(moving), accumulated over 128-row K chunks in PSUM; bias+SiLU fused on the
scalar engine during PSUM eviction.

Group reductions (mean pool, deep-sets mean) are segmented free-axis reduces
on the vector engine. Attention: out = sum_k w[k]*sa[k] with
w[k] = sum_q softmax(S)[q,k], so the [G,G] attention matrix is only needed to
produce per-conformer scalar weights. Scores are computed as 128x128 grams
(4 groups per block, diagonal 32x32 blocks extracted).
"""

import numpy as np
import ml_dtypes

BF16 = ml_dtypes.bfloat16

N = 65536
GROUP = 32
D = 1024
S = 256
V3 = 768
NCORES = 8
NLOC = N // NCORES          # 8192 conformers per core
P = 128
T = 512                     # conformers per tile
SCH = S // P                # 2 scalar-feature chunks
VCH = V3 // P               # 6 vector-feature chunks
CH = SCH + VCH              # 8 chunks of 128 features

# name -> (din, dout)
WSPECS = {
    "ds_phi_s_w1": (S, S), "ds_phi_s_w2": (S, S),
    "ds_phi_v_w1": (V3, V3), "ds_phi_v_w2": (V3, V3),
    "sa_phi_s_w1": (S, S), "sa_phi_s_w2": (S, S),
    "sa_phi_v_w1": (V3, V3), "sa_phi_v_w2": (V3, V3),
    "att_s_w": (S, S), "att_v_w": (V3, V3),
    "ds_rho_s_w1": (S, S), "ds_rho_s_w2": (S, S),
    "ds_rho_v_w1": (V3, V3), "ds_rho_v_w2": (V3, V3),
    "sa_rho_s_w1": (S, S), "sa_rho_s_w2": (S, S),
    "sa_rho_v_w1": (V3, V3), "sa_rho_v_w2": (V3, V3),
}

_CACHE = {}


def _build(nloc):
    import concourse.bass as bass
    from concourse import bacc, mybir, tile

    f32 = mybir.dt.float32
    bf16 = mybir.dt.bfloat16
    AF = mybir.ActivationFunctionType
    AX = mybir.AxisListType

    nt = nloc // T
    gpt = T // GROUP            # groups per tile (16)
    bloc = nloc // GROUP        # molecules per core

    nc = bacc.Bacc(
        "TRN2",
        target_bir_lowering=False,
        debug=False,
        enable_asserts=False,
        num_devices=NCORES,
    )

    xt_d = nc.dram_tensor("xt", [D, nloc], bf16, kind="ExternalInput")
    w_d = {
        n: nc.dram_tensor(f"w_{n}", list(sh), bf16, kind="ExternalInput")
        for n, sh in WSPECS.items()
    }
    b_d = {
        n: nc.dram_tensor(f"b_{n}", [sh[1]], f32, kind="ExternalInput")
        for n, sh in WSPECS.items()
    }
    mask_d = nc.dram_tensor("pool_mask", [P, 4], f32, kind="ExternalInput")
    mp_d = nc.dram_tensor("mpT", [D, bloc], f32, kind="ExternalOutput")
    ds_d = nc.dram_tensor("dsT", [D, bloc], f32, kind="ExternalOutput")
    sa_d = nc.dram_tensor("saT", [D, bloc], f32, kind="ExternalOutput")

    PHI_NAMES = [
        "ds_phi_s_w1", "ds_phi_s_w2", "ds_phi_v_w1", "ds_phi_v_w2",
        "sa_phi_s_w1", "sa_phi_s_w2", "sa_phi_v_w1", "sa_phi_v_w2",
        "att_s_w", "att_v_w",
    ]
    RHO_NAMES = [
        "ds_rho_s_w1", "ds_rho_s_w2", "ds_rho_v_w1", "ds_rho_v_w2",
        "sa_rho_s_w1", "sa_rho_s_w2", "sa_rho_v_w1", "sa_rho_v_w2",
    ]

    with tile.TileContext(nc) as tc:
        with (
            tc.tile_pool(name="weights", bufs=1) as wp,
            tc.tile_pool(name="acc", bufs=1) as accp,
            tc.tile_pool(name="pmm", bufs=4, space=bass.MemorySpace.PSUM) as pmm,
            tc.tile_pool(name="psc", bufs=2, space=bass.MemorySpace.PSUM) as psc,
            tc.tile_pool(name="pwb", bufs=2, space=bass.MemorySpace.PSUM) as pwb,
            tc.tile_pool(name="wdram", bufs=2, space=bass.MemorySpace.DRAM) as wdp,
        ):
            def load_w(pool, name):
                din, dout = WSPECS[name]
                kc = din // P
                wt = pool.tile([P, kc, dout], bf16, tag=f"w_{name}")
                src = w_d[name][:, :].rearrange("(k p) m -> p k m", p=P)
                nc.sync.dma_start(out=wt[:, :, :], in_=src)
                bt = pool.tile([P, dout // P], f32, tag=f"b_{name}")
                bsrc = b_d[name][:].rearrange("(m p) -> p m", p=P)
                nc.sync.dma_start(out=bt[:, :], in_=bsrc)
                return wt, bt

            wtile = {}
            btile = {}
            for nm in PHI_NAMES:
                wtile[nm], btile[nm] = load_w(wp, nm)

            mask_t = wp.tile([P, 4], f32, tag="pool_mask")
            nc.sync.dma_start(out=mask_t[:, :], in_=mask_d[:, :])

            # accumulators: [feature chunk-of-128, chunk index, molecule]
            mp_acc = accp.tile([P, CH, bloc], f32, tag="mp_acc")
            dsm_acc = accp.tile([P, CH, bloc], f32, tag="dsm_acc")
            sap_acc = accp.tile([P, CH, bloc], f32, tag="sap_acc")

            def layer(w, b, in_t, out_t, kc, mc, k_off, m_off, func, n=T):
                """out[:, m_off+m, :] = func(W[:, :, m].T @ in[:, k_off:k_off+kc, :] + b[m])"""
                for m in range(mc):
                    ps = pmm.tile([P, T], f32, tag="mmps")
                    for k in range(kc):
                        nc.tensor.matmul(
                            ps[:, :n],
                            w[:, k, m * P:(m + 1) * P],
                            in_t[:, k_off + k, :],
                            start=(k == 0),
                            stop=(k == kc - 1),
                        )
                    nc.scalar.activation(
                        out=out_t[:, m_off + m, :],
                        in_=ps[:, :n],
                        func=func,
                        bias=b[:, m:m + 1],
                        scale=1.0,
                    )

            def mlp2(pfx, in_t, h1_t, h2_t, n=T):
                layer(wtile[f"{pfx}_s_w1"], btile[f"{pfx}_s_w1"], in_t, h1_t,
                      SCH, SCH, 0, 0, AF.Silu, n)
                layer(wtile[f"{pfx}_v_w1"], btile[f"{pfx}_v_w1"], in_t, h1_t,
                      VCH, VCH, SCH, SCH, AF.Silu, n)
                layer(wtile[f"{pfx}_s_w2"], btile[f"{pfx}_s_w2"], h1_t, h2_t,
                      SCH, SCH, 0, 0, AF.Silu, n)
                layer(wtile[f"{pfx}_v_w2"], btile[f"{pfx}_v_w2"], h1_t, h2_t,
                      VCH, VCH, SCH, SCH, AF.Silu, n)

            xt_all = xt_d[:, :].rearrange("(c p) n -> p c n", p=P)

            with tc.tile_pool(name="work", bufs=2) as work:
                for t in range(nt):
                    n0 = t * T
                    g0 = t * gpt

                    xt = work.tile([P, CH, T], bf16, tag="xt")
                    nc.sync.dma_start(out=xt[:, :, :], in_=xt_all[:, :, n0:n0 + T])

                    # mean-pool accumulator (sums; host divides by GROUP)
                    for c in range(CH):
                        nc.vector.reduce_sum(
                            out=mp_acc[:, c, g0:g0 + gpt],
                            in_=xt[:, c, :].rearrange("p (g j) -> p g j", j=GROUP),
                            axis=AX.X,
                        )

                    # deep sets phi
                    ds_h1 = work.tile([P, CH, T], bf16, tag="ds_h1")
                    ds_h2 = work.tile([P, CH, T], bf16, tag="ds_h2")
                    mlp2("ds_phi", xt, ds_h1, ds_h2)
                    for c in range(CH):
                        nc.vector.reduce_sum(
                            out=dsm_acc[:, c, g0:g0 + gpt],
                            in_=ds_h2[:, c, :].rearrange("p (g j) -> p g j", j=GROUP),
                            axis=AX.X,
                        )

                    # self-attention phi
                    sa_h1 = work.tile([P, CH, T], bf16, tag="sa_h1")
                    sa = work.tile([P, CH, T], bf16, tag="sa")
                    mlp2("sa_phi", xt, sa_h1, sa)

                    # attention scores linear (Identity keeps bias-add, casts bf16)
                    sct = work.tile([P, CH, T], bf16, tag="sct")
                    layer(wtile["att_s_w"], btile["att_s_w"], sa, sct,
                          SCH, SCH, 0, 0, AF.Identity)
                    layer(wtile["att_v_w"], btile["att_v_w"], sa, sct,
                          VCH, VCH, SCH, SCH, AF.Identity)

                    # gram scores: 4 groups per 128-column block, diag blocks valid
                    sc_c = work.tile([P, 4, GROUP], f32, tag="sc_c")
                    for blk in range(4):
                        ps_sc = psc.tile([P, P], f32, tag="scps")
                        col = blk * P
                        for c in range(CH):
                            nc.tensor.matmul(
                                ps_sc[:, :],
                                sct[:, c, col:col + P],
                                sct[:, c, col:col + P],
                                start=(c == 0),
                                stop=(c == CH - 1),
                            )
                        for j in range(4):
                            r0 = j * GROUP
                            nc.vector.tensor_copy(
                                out=sc_c[r0:r0 + GROUP, blk, :],
                                in_=ps_sc[r0:r0 + GROUP, r0:r0 + GROUP],
                            )

                    # softmax without max-subtraction (|scores| <~ 6)
                    ex = work.tile([P, 4, GROUP], f32, tag="ex")
                    nc.scalar.activation(out=ex[:, :, :], in_=sc_c[:, :, :], func=AF.Exp)
                    ssum = work.tile([P, 4], f32, tag="ssum")
                    nc.vector.reduce_sum(out=ssum[:, :], in_=ex[:, :, :], axis=AX.X)
                    rs = work.tile([P, 4], f32, tag="rs")
                    nc.vector.reciprocal(out=rs[:, :], in_=ssum[:, :])

                    # per-conformer weights w[k] = sum_q attn[q, k]
                    wd_t = wdp.tile([4 * 4, GROUP], bf16, tag="wd")
                    for blk in range(4):
                        rpool = work.tile([P, 4], f32, tag="rpool", bufs=8)
                        nc.vector.tensor_scalar_mul(
                            rpool[:, :], mask_t[:, :], rs[:, blk:blk + 1]
                        )
                        wps = pwb.tile([4, GROUP], f32, tag="wps")
                        nc.tensor.matmul(
                            wps[:, :], rpool[:, :], ex[:, blk, :],
                            start=True, stop=True,
                        )
                        wrow = work.tile([4, GROUP], bf16, tag="wrow", bufs=8)
                        nc.vector.tensor_copy(out=wrow[:, :], in_=wps[:, :])
                        nc.gpsimd.dma_start(
                            out=wd_t[blk * 4:blk * 4 + 4, :], in_=wrow[:, :]
                        )

                    # broadcast w row [1, T] across all 128 partitions
                    wbc = work.tile([P, T], bf16, tag="wbc")
                    wd_ap = wd_t[:, :]
                    bcast = bass.AP(
                        tensor=wd_ap.tensor,
                        offset=wd_ap.offset,
                        ap=[[0, P]] + list(wd_ap.ap),
                    )
                    nc.gpsimd.dma_start(out=wbc[:, :], in_=bcast)

                    # weighted group-sum of sa
                    for c in range(CH):
                        wtmp = work.tile([P, T], bf16, tag="wtmp")
                        nc.vector.tensor_mul(wtmp[:, :], sa[:, c, :], wbc[:, :])
                        nc.vector.reduce_sum(
                            out=sap_acc[:, c, g0:g0 + gpt],
                            in_=wtmp[:, :].rearrange("p (g j) -> p g j", j=GROUP),
                            axis=AX.X,
                        )

            nc.sync.dma_start(
                out=mp_d[:, :].rearrange("(c p) n -> p c n", p=P),
                in_=mp_acc[:, :, :],
            )

            # rho MLPs on pooled values ([*, bloc] wide)
            with tc.tile_pool(name="rho", bufs=1) as rp:
                for nm in RHO_NAMES:
                    wtile[nm], btile[nm] = load_w(rp, nm)

                dsm_bf = rp.tile([P, CH, bloc], bf16, tag="dsm_bf")
                nc.vector.tensor_copy(out=dsm_bf[:, :, :], in_=dsm_acc[:, :, :])
                sap_bf = rp.tile([P, CH, bloc], bf16, tag="sap_bf")
                nc.vector.tensor_copy(out=sap_bf[:, :, :], in_=sap_acc[:, :, :])

                ds_h1r = rp.tile([P, CH, bloc], bf16, tag="ds_h1r")
                ds_out = rp.tile([P, CH, bloc], f32, tag="ds_out")
                layer(wtile["ds_rho_s_w1"], btile["ds_rho_s_w1"], dsm_bf, ds_h1r,
                      SCH, SCH, 0, 0, AF.Silu, n=bloc)
                layer(wtile["ds_rho_v_w1"], btile["ds_rho_v_w1"], dsm_bf, ds_h1r,
                      VCH, VCH, SCH, SCH, AF.Silu, n=bloc)
                layer(wtile["ds_rho_s_w2"], btile["ds_rho_s_w2"], ds_h1r, ds_out,
                      SCH, SCH, 0, 0, AF.Silu, n=bloc)
                layer(wtile["ds_rho_v_w2"], btile["ds_rho_v_w2"], ds_h1r, ds_out,
                      VCH, VCH, SCH, SCH, AF.Silu, n=bloc)

                sa_h1r = rp.tile([P, CH, bloc], bf16, tag="sa_h1r")
                sa_out = rp.tile([P, CH, bloc], f32, tag="sa_out")
                layer(wtile["sa_rho_s_w1"], btile["sa_rho_s_w1"], sap_bf, sa_h1r,
                      SCH, SCH, 0, 0, AF.Silu, n=bloc)
                layer(wtile["sa_rho_v_w1"], btile["sa_rho_v_w1"], sap_bf, sa_h1r,
                      VCH, VCH, SCH, SCH, AF.Silu, n=bloc)
                layer(wtile["sa_rho_s_w2"], btile["sa_rho_s_w2"], sa_h1r, sa_out,
                      SCH, SCH, 0, 0, AF.Silu, n=bloc)
                layer(wtile["sa_rho_v_w2"], btile["sa_rho_v_w2"], sa_h1r, sa_out,
                      VCH, VCH, SCH, SCH, AF.Silu, n=bloc)

                nc.sync.dma_start(
                    out=ds_d[:, :].rearrange("(c p) n -> p c n", p=P),
                    in_=ds_out[:, :, :],
                )
                nc.sync.dma_start(
                    out=sa_d[:, :].rearrange("(c p) n -> p c n", p=P),
                    in_=sa_out[:, :, :],
                )

    nc.compile()
    return nc


def _get_nc(nloc=NLOC):
    if nloc not in _CACHE:
        _CACHE[nloc] = _build(nloc)
    return _CACHE[nloc]


def _prep_shared(params):
    """bf16 weights (rho-mean scaling folded in) + f32 biases + pool mask."""
    def get(name):
        parts = name.split("_")
        leaf = parts[-1]                       # w1/w2/w/b
        blk = "_".join(parts[:-1])             # e.g. ds_phi_s, att_s
        return np.asarray(params[blk][leaf], np.float32)

    shared = {}
    for n in WSPECS:
        w = get(n)
        if n in ("ds_rho_s_w1", "ds_rho_v_w1"):
            w = w / np.float32(GROUP)          # fold the deep-sets group mean
        shared[f"w_{n}"] = np.ascontiguousarray(w.astype(BF16))
        # biases: w1 -> b1, w2 -> b2, w -> b
        leaf = n.split("_")[-1]
        bleaf = "b" if leaf == "w" else "b" + leaf[1:]
        blk = "_".join(n.split("_")[:-1])
        shared[f"b_{n}"] = np.ascontiguousarray(
            np.asarray(params[blk][bleaf], np.float32)
        )
    mask = np.zeros((P, 4), np.float32)
    for q in range(P):
        mask[q, q // GROUP] = 1.0
    shared["pool_mask"] = mask
    return shared


def _run(x, params, nloc=NLOC, trace=False):
    from concourse.bass_utils import run_bass_kernel_spmd

    nc = _get_nc(nloc)
    shared = _prep_shared(params)
    in_maps = []
    for c in range(NCORES):
        xs = x[c * nloc:(c + 1) * nloc]
        xt = np.ascontiguousarray(xs.T.astype(BF16))
        m = dict(shared)
        m["xt"] = xt
        in_maps.append(m)
    res = run_bass_kernel_spmd(
        nc, in_maps, list(range(NCORES)), trace=trace
    )
    return res


def _assemble(res):
    mp = np.concatenate([r["mpT"].T for r in res.results], axis=0)
    mp = mp * np.float32(1.0 / GROUP)
    ds = np.concatenate([r["dsT"].T for r in res.results], axis=0)
    sa = np.concatenate([r["saT"].T for r in res.results], axis=0)
    nmol = mp.shape[0]
    return (
        np.ascontiguousarray(mp[:, :S]),
        np.ascontiguousarray(mp[:, S:]).reshape(nmol, S, 3),
        np.ascontiguousarray(ds[:, :S]),
        np.ascontiguousarray(ds[:, S:]).reshape(nmol, S, 3),
        np.ascontiguousarray(sa[:, :S]),
        np.ascontiguousarray(sa[:, S:]).reshape(nmol, S, 3),
    )


def kernel(conformer_embeddings, batch_indices, params):
    # batch_indices are sorted uniform groups of 32 (the reference ignores
    # them too - it pools by fixed reshape), so they don't affect the result.
    x = np.asarray(conformer_embeddings, np.float32)
    res = _run(x, params)
    return _assemble(res)
